# revision 44
# baseline (speedup 1.0000x reference)
"""DiffSAGE GNN layer on 8 Trainium2 NeuronCores — v2.

Math (per reference):
    msg      = x[src] - x[dst]                      # per edge
    agg      = segment_mean(msg, dst, N)            # zeros where cnt==0
    out      = agg @ Wl.T + bl + x @ Wr.T

Identity used by the kernel:
    sum_{e: dst=i} (x[src_e] - x[i]) = gsum[i] - cnt[i]*x[i]
    agg[i] = gsum[i] * r[i] - x[i] * s[i]
      where r = 1/max(cnt,1), s = cnt*r  (0 or 1).  r and x*s are computed
      on HOST and shipped as slabs, so the device never touches counts.

Distribution: destination-node sharding across the 8 cores (no collectives).

v2 changes vs v1:
  - nq=4 SWDGE queues; per-(window,side) gather calls (G=1) with trailing
    pad indices marked -1 (HW skips them -> desc count ~= real edges).
  - cnt matmul removed; r pre-replicated [D, NSLAB] bf16 slab from host.
  - one-hot built for NB=8 chunks per DVE op (amortizes fixed cost).
  - weights / x slabs / x*s slab in bf16 -> bf16 matmuls everywhere.

v3 changes vs v2 (measured bottleneck: Q7 SWDGE descriptor GENERATION,
~9.25ns/desc serial per queue, 4 queues in parallel; transfer+compute hide
under it):
  - SPLIT 32767 -> 25000: the A/B table split carries equal edge mass, so
    the 4 SWDGE queues (A-calls on even, B-calls on odd) see equal
    descriptor load. The old split put 65% of gen work on half the queues.
  - flat window packing: nodes are greedily packed into windows so every
    window-side holds <= 1024 edges (capa = capb = 8 chunks guaranteed,
    ~99.5% descriptor fill, uniform per-call gen cost). The node->slab
    position map is undone on the host at unshard time.
  - one-hot comparands (dstr slots + iota) in bf16 instead of f32: 2x DVE
    throughput on the is_equal ops, half the dstr DMA bytes.
  - stage pool depth 4: two gather calls in flight per queue.
"""

import sys

import numpy as np

try:
    import concourse.bass as bass
except Exception:  # pragma: no cover - harness path setup
    for p in (
        "/root/.axon_site",
        "/root/.axon_site/_ro/trn_rl_repo",
        "/root/.axon_site/_ro/pypackages",
        "/opt/trn_rl_repo",
    ):
        if p not in sys.path:
            sys.path.append(p)
    import concourse.bass as bass

from contextlib import ExitStack

import ml_dtypes

import concourse.mybir as mybir
import concourse.tile as tile
from concourse import bacc, bass_utils

F32 = mybir.dt.float32
BF16 = mybir.dt.bfloat16
I16 = mybir.dt.int16
I32 = mybir.dt.int32

D = 128          # feature dim (in and out)
WN = 64          # nodes per window
CHUNK = 128      # edges per matmul chunk (contraction dim)
SPLIT = 25000    # x rows < SPLIT go to table A, rest to table B.
                 # 25000 balances the A/B edge mass 50/50 so the four SWDGE
                 # queues (A on even, B on odd) carry equal descriptor load;
                 # the old 32767 (int16 max) put 65% of descs on the A queues.
NB = 8           # chunks per one-hot DVE op
STAGE_BUFS = 4   # stage pool depth (first STAGE_BUFS windows keep 0-pads)


class Cfg:
    def __init__(self, nta, ntb, wins, capa, capb, n_cores, group=1):
        self.NTA = nta          # rows in table A (incl zero row)
        self.NTB = ntb          # rows in table B (incl zero row)
        self.WINS = wins        # 64-node windows per core
        self.CAPA = capa        # A-chunks per window
        self.CAPB = capb        # B-chunks per window
        self.NSLAB = wins * WN  # padded nodes per core
        self.N_CORES = n_cores
        self.G = group
        assert wins % group == 0


def build_nc(cfg: Cfg, repeat: int = 1, nq: int = 4,
             skip_gather: bool = False, skip_compute: bool = False,
             single_packet: bool = False, stage_bufs: int = STAGE_BUFS,
             io_bufs: int = 3, oh_bufs: int = 4, pacc_bufs: int = 2,
             halfcalls: bool = False, ind_ka: int = 0) -> bass.Bass:
    nc = bacc.Bacc("TRN2", num_swdge_queues=nq)
    WINS, CAPA, CAPB, NSLAB, G = cfg.WINS, cfg.CAPA, cfg.CAPB, cfg.NSLAB, cfg.G
    CAP = CAPA + CAPB
    CAPQ = NB * ((CAP + NB - 1) // NB)
    NCALLS = WINS // G
    if ind_ka:
        assert G == 1 and not halfcalls and 0 < ind_ka < CAPA
    NIA = G * (CAPA - ind_ka) * CHUNK   # idxs per A-call (swdge part)
    NIB = G * CAPB * CHUNK

    taba = nc.dram_tensor("taba", [cfg.NTA, D], BF16, kind="ExternalInput")
    tabb = nc.dram_tensor("tabb", [cfg.NTB, D], BF16, kind="ExternalInput")
    if halfcalls:
        assert CAPA % 2 == 0 and CAPB % 2 == 0
        idxa = nc.dram_tensor("idxa", [NCALLS, CHUNK, 2, NIA // 32], I16,
                              kind="ExternalInput")
        idxb = nc.dram_tensor("idxb", [NCALLS, CHUNK, 2, NIB // 32], I16,
                              kind="ExternalInput")
        cnts = nc.dram_tensor("cnts", [1, NCALLS, 4], I32, kind="ExternalInput")
    else:
        idxa = nc.dram_tensor("idxa", [NCALLS, CHUNK, NIA // 16], I16,
                              kind="ExternalInput")
        idxb = nc.dram_tensor("idxb", [NCALLS, CHUNK, NIB // 16], I16,
                              kind="ExternalInput")
        cnts = nc.dram_tensor("cnts", [1, NCALLS, 2], I32, kind="ExternalInput")
    xwd = nc.dram_tensor("xwd", [D, NSLAB], BF16, kind="ExternalInput")
    xsd = nc.dram_tensor("xsd", [D, NSLAB], BF16, kind="ExternalInput")
    rbd = nc.dram_tensor("rbd", [D, NSLAB], BF16, kind="ExternalInput")
    dstr = nc.dram_tensor("dstr", [WINS, CHUNK, CAPQ], BF16, kind="ExternalInput")
    ioffa = (nc.dram_tensor("ioffa", [WINS, CHUNK, ind_ka], I32,
                            kind="ExternalInput") if ind_ka else None)
    wlt = nc.dram_tensor("wlt", [D, D], BF16, kind="ExternalInput")
    wrt = nc.dram_tensor("wrt", [D, D], BF16, kind="ExternalInput")
    blb = nc.dram_tensor("blb", [D, 1], F32, kind="ExternalInput")
    outh = nc.dram_tensor("out", [D, NSLAB], F32, kind="ExternalOutput")

    with ExitStack() as ctx:
        tc = ctx.enter_context(tile.TileContext(nc))
        singles = ctx.enter_context(tc.tile_pool(name="singles", bufs=1))
        stage_p = ctx.enter_context(tc.tile_pool(name="stage", bufs=stage_bufs))
        io_p = ctx.enter_context(tc.tile_pool(name="io", bufs=io_bufs))
        oh_p = ctx.enter_context(tc.tile_pool(name="oh", bufs=oh_bufs))
        wrk = ctx.enter_context(tc.tile_pool(name="wrk", bufs=3))
        pacc = ctx.enter_context(tc.tile_pool(name="pacc", bufs=pacc_bufs, space="PSUM"))
        plin = ctx.enter_context(tc.tile_pool(name="plin", bufs=2, space="PSUM"))

        # ---- one-time constants ----
        xw_sb = singles.tile([D, NSLAB], BF16)
        nc.sync.dma_start(out=xw_sb[:], in_=xwd[:])
        xs_sb = singles.tile([D, NSLAB], BF16)
        nc.sync.dma_start(out=xs_sb[:], in_=xsd[:])
        rb_sb = singles.tile([D, NSLAB], BF16)
        nc.sync.dma_start(out=rb_sb[:], in_=rbd[:])
        wlt_sb = singles.tile([D, D], BF16)
        nc.sync.dma_start(out=wlt_sb[:], in_=wlt[:])
        wrt_sb = singles.tile([D, D], BF16)
        nc.sync.dma_start(out=wrt_sb[:], in_=wrt[:])
        bl_sb = singles.tile([D, 1], F32)
        nc.sync.dma_start(out=bl_sb[:], in_=blb[:])
        cnt_sb = singles.tile([1, NCALLS, 4 if halfcalls else 2], I32)
        nc.sync.dma_start(out=cnt_sb[:], in_=cnts[:])
        iota_i = singles.tile([CHUNK, NB, WN], I32)
        nc.gpsimd.iota(iota_i[:], pattern=[[0, NB], [1, WN]], channel_multiplier=0)
        # bf16 comparands (exact for 0..63 and the 999->1000 pad sentinel)
        # double DVE throughput on the one-hot is_equal ops vs f32.
        iota8 = singles.tile([CHUNK, NB, WN], BF16)
        nc.vector.tensor_copy(out=iota8[:], in_=iota_i[:])

        # ring of Pool registers for per-call gather counts.  Pool SEQ can
        # only run ahead of the engine by its wait+exec queue depth (~8
        # instructions), so a ring of 16 is safely WAR-hazard free.
        NREGS = 16
        cnt_regs = [nc.gpsimd.alloc_register(f"cntreg{i}") for i in range(NREGS)]
        reg_i = [0]

        def next_cnt_reg(cnt_ap):
            r = cnt_regs[reg_i[0] % NREGS]
            reg_i[0] += 1
            nc.gpsimd.reg_load(r, cnt_ap)
            return r

        state = {}

        def window_body(w):
            g, wg = divmod(w, G)
            if wg == 0 and halfcalls:
                ia_t = io_p.tile([CHUNK, 2, NIA // 32], I16, tag="ia")
                nc.sync.dma_start(out=ia_t[:], in_=idxa[g])
                ib_t = io_p.tile([CHUNK, 2, NIB // 32], I16, tag="ib")
                nc.sync.dma_start(out=ib_t[:], in_=idxb[g])
                stga = stage_p.tile([CHUNK, G * CAPA, D], BF16, tag="stga")
                stgb = stage_p.tile([CHUNK, G * CAPB, D], BF16, tag="stgb")
                if skip_gather:
                    nc.vector.memset(stga[:, 0, :], 0)
                    nc.vector.memset(stgb[:, 0, :], 0)
                else:
                    ha, hb = CAPA // 2, CAPB // 2
                    for h in range(2):
                        r = next_cnt_reg(cnt_sb[0:1, g, h : h + 1])
                        nc.gpsimd.dma_gather(
                            stga[:, h * ha : (h + 1) * ha, :], taba[:],
                            ia_t[:, h, :], NIA // 2, r, D,
                            single_packet=single_packet,
                            queue_num=(4 * g + h) % nq,
                        )
                    for h in range(2):
                        r = next_cnt_reg(cnt_sb[0:1, g, 2 + h : 3 + h])
                        nc.gpsimd.dma_gather(
                            stgb[:, h * hb : (h + 1) * hb, :], tabb[:],
                            ib_t[:, h, :], NIB // 2, r, D,
                            single_packet=single_packet,
                            queue_num=(4 * g + 2 + h) % nq,
                        )
                state["stga"] = stga
                state["stgb"] = stgb
            elif wg == 0:
                ia_t = io_p.tile([CHUNK, NIA // 16], I16, tag="ia")
                nc.sync.dma_start(out=ia_t[:], in_=idxa[g])
                ib_t = io_p.tile([CHUNK, NIB // 16], I16, tag="ib")
                nc.sync.dma_start(out=ib_t[:], in_=idxb[g])
                stga = stage_p.tile([CHUNK, G * CAPA, D], BF16, tag="stga")
                if skip_gather:
                    nc.vector.memset(stga[:, 0, :], 0)
                else:
                    na_reg = next_cnt_reg(cnt_sb[0:1, g, 0:1])
                    ga_out = stga[:, 0 : CAPA - ind_ka, :] if ind_ka else stga[:]
                    nc.gpsimd.dma_gather(
                        ga_out, taba[:], ia_t[:], NIA, na_reg, D,
                        single_packet=single_packet, queue_num=(2 * g) % nq,
                    )
                    if ind_ka:
                        iof_t = io_p.tile([CHUNK, ind_ka], I32, tag="iofa")
                        nc.sync.dma_start(out=iof_t[:], in_=ioffa[g])
                        nc.gpsimd.indirect_dma_start(
                            out=stga[:, CAPA - ind_ka : CAPA, :],
                            out_offset=None,
                            in_=taba[:],
                            in_offset=bass.IndirectOffsetOnAxis(
                                ap=iof_t[:, 0:ind_ka], axis=0,
                            ),
                        )
                state["stga"] = stga
                stgb = stage_p.tile([CHUNK, G * CAPB, D], BF16, tag="stgb")
                if skip_gather:
                    nc.vector.memset(stgb[:, 0, :], 0)
                else:
                    nb_reg = next_cnt_reg(cnt_sb[0:1, g, 1:2])
                    nc.gpsimd.dma_gather(
                        stgb[:], tabb[:], ib_t[:], NIB, nb_reg, D,
                        single_packet=single_packet, queue_num=(2 * g + 1) % nq,
                    )
                state["stgb"] = stgb

            if skip_compute:
                return
            dst_t = io_p.tile([CHUNK, CAPQ], BF16, tag="dst")
            nc.sync.dma_start(out=dst_t[:], in_=dstr[w])

            agg_ps = pacc.tile([D, WN], F32, space="PSUM", tag="agg")
            for c in range(CAP):
                if c % NB == 0:
                    oh8 = oh_p.tile([CHUNK, NB, WN], BF16, tag="oh")
                    nc.vector.tensor_tensor(
                        out=oh8[:],
                        in0=dst_t[:, c : c + NB].to_broadcast([CHUNK, NB, WN]),
                        in1=iota8[:],
                        op=mybir.AluOpType.is_equal,
                    )
                    state["oh8"] = oh8
                if c < CAPA:
                    msg = state["stga"][:, wg * CAPA + c, :]
                else:
                    msg = state["stgb"][:, wg * CAPB + (c - CAPA), :]
                nc.tensor.matmul(
                    agg_ps[:], lhsT=msg, rhs=state["oh8"][:, c % NB, :],
                    start=(c == 0), stop=(c == CAP - 1),
                )

            # ---- window epilogue ----
            sl = slice(w * WN, (w + 1) * WN)
            t1 = wrk.tile([D, WN], BF16, tag="t1")
            nc.vector.tensor_tensor(
                out=t1[:], in0=agg_ps[:], in1=rb_sb[:, sl], op=mybir.AluOpType.mult
            )
            agg2 = wrk.tile([D, WN], BF16, tag="agg2")
            nc.vector.tensor_tensor(
                out=agg2[:], in0=t1[:], in1=xs_sb[:, sl],
                op=mybir.AluOpType.subtract,
            )

            lin = plin.tile([D, WN], F32, space="PSUM", tag="lin")
            nc.tensor.matmul(lin[:], lhsT=wlt_sb[:], rhs=agg2[:], start=True, stop=False)
            nc.tensor.matmul(lin[:], lhsT=wrt_sb[:], rhs=xw_sb[:, sl], start=False, stop=True)

            # bias add on the ACT engine; store transposed [D, node] --
            # the host transposes back for free.
            outt = wrk.tile([D, WN], F32, tag="outt")
            nc.scalar.activation(
                outt[:], lin[:], mybir.ActivationFunctionType.Identity,
                bias=bl_sb[:, 0:1],
            )
            nc.sync.dma_start(out=outh[:, sl], in_=outt[:])

        if repeat > 1:
            rep_start = nc.snap(0)
            rep_end = nc.snap(repeat)
            with tc.For_i(rep_start, rep_end, 1, name="rep") as _rep_i:
                for w in range(WINS):
                    window_body(w)
        else:
            for w in range(WINS):
                window_body(w)

    nc.compile()
    return nc


def wrap_idx(idx):
    """[n] -> [128, n/16] int16 (16-partition wrap, replicated 8x)."""
    n = len(idx)
    w = idx.reshape(n // 16, 16)
    return np.ascontiguousarray(np.tile(w.T, (8, 1)))


def pack_side(gidx, win, wins, cap, order_key):
    """Pack one side's edges (gather indices `gidx`, window ids `win`) into
    per-window slot arrays.  Returns (slots [wins, cap*128] int16 gather idx,
    flat positions of real edges, their order)."""
    slots = cap * CHUNK
    counts = np.bincount(win, minlength=wins)
    assert counts.max() <= slots, (counts.max(), slots)
    order = np.lexsort((gidx, win))
    starts = np.zeros(wins, dtype=np.int64)
    starts[1:] = np.cumsum(counts)[:-1]
    rank = np.arange(len(gidx)) - starts[win[order]]
    flat_pos = win[order] * slots + rank
    arr = np.zeros(wins * slots, dtype=np.int16)  # pad -> zero row
    arr[flat_pos] = gidx[order].astype(np.int16)
    return arr, flat_pos, order


def neg_tail_pads(arr_calls, skip_first=STAGE_BUFS):
    """Mark trailing pad slots (0) of each gather call as -1 (skipped by HW)
    and return the per-call count of remaining (non-negative) indices, which
    MUST be passed as num_idxs_reg (the ucode contract).

    The first `skip_first` calls keep 0-pads: they fully initialize the
    cycling stage buffers so later skipped slots only ever alias finite
    stale data, never uninitialized SBUF."""
    counts = np.empty(len(arr_calls), dtype=np.int32)
    for i, a in enumerate(arr_calls):
        if i < skip_first:
            counts[i] = len(a)
            continue
        nz = np.nonzero(a)[0]
        end = (int(nz[-1]) + 1) if len(nz) else 1  # keep >= 1 real idx
        a[end:] = -1
        counts[i] = end
    return counts


def run_graph(x, edge_index, Wl, bl, Wr, n_cores=8, group=1, trace=False,
              min_capa=1, min_capb=1, repeat=1, nq=4, skip_gather=False,
              skip_compute=False, single_packet=False, neg_pads=True,
              stage_bufs=STAGE_BUFS, split=SPLIT, io_bufs=3, oh_bufs=4,
              pacc_bufs=2, halfcalls=False, ind_ka=0, flatpack=True,
              budget=8 * CHUNK):
    """Full pipeline: host prep -> one SPMD compile -> run -> unshard."""
    x = np.asarray(x, dtype=np.float32)
    n, d = x.shape
    assert d == D
    src = np.asarray(edge_index[0], dtype=np.int64)
    dst = np.asarray(edge_index[1], dtype=np.int64)
    assert n % n_cores == 0
    npc = n // n_cores
    is_a = src < split

    core_of = dst // npc
    ldst_all = dst - core_of * npc

    if flatpack:
        # Greedy node->window packing: close a window when adding the next
        # node would push either side past `budget` edges or 64 nodes.
        # Guarantees capa = capb = budget/128 with ~99.5% descriptor fill.
        da = np.bincount(dst[is_a], minlength=n)
        db = np.bincount(dst[~is_a], minlength=n)
        win_of = np.empty(n, dtype=np.int64)
        rel_of = np.empty(n, dtype=np.int64)
        wins_per_core = []
        for c in range(n_cores):
            lo = c * npc
            w = 0
            asum = bsum = nn = 0
            for i in range(lo, lo + npc):
                if (nn == WN or asum + da[i] > budget or bsum + db[i] > budget):
                    w += 1
                    asum = bsum = nn = 0
                win_of[i] = w
                rel_of[i] = nn
                asum += da[i]
                bsum += db[i]
                nn += 1
            wins_per_core.append(w + 1)
        wins = max(wins_per_core)
        while wins % group:
            wins += 1
        win_all = core_of * wins + win_of[dst]
        rel_all = rel_of[dst]
    else:
        wins = -(-npc // WN)
        while wins % group:
            wins += 1
        win_of = (np.arange(n) % npc) // WN
        rel_of = (np.arange(n) % npc) % WN
        win_all = core_of * wins + ldst_all // WN
        rel_all = ldst_all % WN
    nslab = wins * WN

    # capacities from global max window occupancy (uniform across cores)
    ca = np.bincount(win_all[is_a], minlength=n_cores * wins)
    cb = np.bincount(win_all[~is_a], minlength=n_cores * wins)
    capa = max(int(min_capa), int(-(-ca.max() // CHUNK)))
    capb = max(int(min_capb), int(-(-cb.max() // CHUNK)))
    if halfcalls:  # equal chunk-aligned halves per side
        capa += capa % 2
        capb += capb % 2
    na = min(n, split)
    ntb = max(n - split, 1) + 1
    cfg = Cfg(nta=na + 1, ntb=ntb, wins=wins, capa=capa, capb=capb,
              n_cores=n_cores, group=group)

    # shared tables (row 0 = zeros for padding)
    taba = np.zeros((na + 1, D), dtype=ml_dtypes.bfloat16)
    taba[1 : na + 1] = x[:na].astype(ml_dtypes.bfloat16)
    tabb = np.zeros((ntb, D), dtype=ml_dtypes.bfloat16)
    if n > split:
        tabb[1 : n - split + 1] = x[split:].astype(ml_dtypes.bfloat16)
    wlt = np.ascontiguousarray(np.asarray(Wl, np.float32).T).astype(ml_dtypes.bfloat16)
    wrt = np.ascontiguousarray(np.asarray(Wr, np.float32).T).astype(ml_dtypes.bfloat16)
    blb = np.ascontiguousarray(np.asarray(bl, np.float32).reshape(D, 1))

    cnt = np.bincount(dst, minlength=n).astype(np.float32)
    r_all = 1.0 / np.maximum(cnt, 1.0)
    s_all = np.minimum(cnt, 1.0)

    if ind_ka:
        assert group == 1 and not halfcalls and 0 < ind_ka < capa
    ncalls = wins // group
    nia, nib = group * (capa - ind_ka) * CHUNK, group * capb * CHUNK
    in_maps = []
    pos_maps = []
    for c in range(n_cores):
        m = core_of == c
        ms, ma = src[m], is_a[m]
        mwin = win_all[m] - c * wins
        mrel = rel_all[m].astype(np.float32)

        sa, posa, orda = pack_side(ms[ma] + 1, mwin[ma], wins, capa, None)
        sb, posb, ordb = pack_side(ms[~ma] - (split - 1), mwin[~ma], wins, capb, None)

        # combined dst_rel slots: [wins, capq*128], pad 999
        capq = NB * ((capa + capb + NB - 1) // NB)
        dstrv = np.full((wins, capq * CHUNK), 999.0, dtype=np.float32)
        wa, ra = np.divmod(posa, capa * CHUNK)
        dstrv[wa, ra] = mrel[ma][orda]
        wb, rb = np.divmod(posb, capb * CHUNK)
        dstrv[wb, capa * CHUNK + rb] = mrel[~ma][ordb]

        # peel the indirect-DMA offload region (last ind_ka chunks of A)
        if ind_ka:
            sa_full = sa.reshape(wins, capa * CHUNK)
            ioff = sa_full[:, (capa - ind_ka) * CHUNK :].astype(np.int32)
            ioffa_arr = np.ascontiguousarray(
                ioff.reshape(wins, ind_ka, CHUNK).transpose(0, 2, 1)
            )
            sa = np.ascontiguousarray(sa_full[:, : (capa - ind_ka) * CHUNK])
        # gather call arrays: [ncalls, group*cap*128]; trailing pads -> -1
        if halfcalls:
            sa_c = sa.reshape(ncalls * 2, nia // 2)
            sb_c = sb.reshape(ncalls * 2, nib // 2)
            if neg_pads and not skip_gather:
                ca_cnt = neg_tail_pads(sa_c, skip_first=2 * stage_bufs)
                cb_cnt = neg_tail_pads(sb_c, skip_first=2 * stage_bufs)
            else:
                ca_cnt = np.full(ncalls * 2, nia // 2, dtype=np.int32)
                cb_cnt = np.full(ncalls * 2, nib // 2, dtype=np.int32)
            cnts_arr = np.concatenate(
                [ca_cnt.reshape(ncalls, 2), cb_cnt.reshape(ncalls, 2)], axis=1
            )[None]  # [1, ncalls, 4]
            # [ncalls, 128, 2, (nia/2)/16]
            ia_w = np.stack([wrap_idx(a) for a in sa_c]).reshape(
                ncalls, 2, CHUNK, nia // 32).transpose(0, 2, 1, 3)
            ib_w = np.stack([wrap_idx(b) for b in sb_c]).reshape(
                ncalls, 2, CHUNK, nib // 32).transpose(0, 2, 1, 3)
            ia_w = np.ascontiguousarray(ia_w)
            ib_w = np.ascontiguousarray(ib_w)
        else:
            sa_c = sa.reshape(ncalls, nia)
            sb_c = sb.reshape(ncalls, nib)
            if neg_pads and not skip_gather:
                ca_cnt = neg_tail_pads(sa_c, skip_first=stage_bufs)
                cb_cnt = neg_tail_pads(sb_c, skip_first=stage_bufs)
            else:
                ca_cnt = np.full(ncalls, nia, dtype=np.int32)
                cb_cnt = np.full(ncalls, nib, dtype=np.int32)
            cnts_arr = np.stack([ca_cnt, cb_cnt], axis=1)[None]  # [1, ncalls, 2]
            ia_w = np.stack([wrap_idx(a) for a in sa_c])
            ib_w = np.stack([wrap_idx(b) for b in sb_c])

        # dst tile layout: [wins, 128, capq] with slot c*128+p -> [p, c]
        dstr_t = np.ascontiguousarray(
            dstrv.reshape(wins, capq, CHUNK).transpose(0, 2, 1)
        ).astype(ml_dtypes.bfloat16)

        lsl = slice(c * npc, (c + 1) * npc)
        pos = win_of[lsl] * WN + rel_of[lsl]  # slab column of each local node
        pos_maps.append(pos)
        xw = np.zeros((D, nslab), dtype=ml_dtypes.bfloat16)
        xw[:, pos] = x[lsl].T.astype(ml_dtypes.bfloat16)
        xs = np.zeros((D, nslab), dtype=ml_dtypes.bfloat16)
        xs[:, pos] = (x[lsl] * s_all[lsl, None]).T.astype(ml_dtypes.bfloat16)
        rbv = np.zeros((nslab,), dtype=np.float32)
        rbv[pos] = r_all[lsl]
        rb = np.ascontiguousarray(
            np.broadcast_to(rbv[None, :], (D, nslab))
        ).astype(ml_dtypes.bfloat16)

        im = {
            "taba": taba,
            "tabb": tabb,
            "idxa": ia_w,
            "idxb": ib_w,
            "cnts": cnts_arr,
            "xwd": xw,
            "xsd": xs,
            "rbd": rb,
            "dstr": dstr_t,
            "wlt": wlt,
            "wrt": wrt,
            "blb": blb,
        }
        if ind_ka:
            im["ioffa"] = ioffa_arr
        in_maps.append(im)

    global LAST_POS
    LAST_POS = pos_maps
    nc = build_nc(cfg, repeat=repeat, nq=nq, skip_gather=skip_gather,
                  skip_compute=skip_compute, single_packet=single_packet,
                  stage_bufs=stage_bufs, io_bufs=io_bufs, oh_bufs=oh_bufs,
                  pacc_bufs=pacc_bufs, halfcalls=halfcalls, ind_ka=ind_ka)
    res = bass_utils.run_bass_kernel_spmd(
        nc, in_maps, core_ids=list(range(n_cores)), trace=trace
    )
    out = np.concatenate(
        [res.results[c]["out"][:, pos_maps[c]].T for c in range(n_cores)], axis=0
    )
    return np.ascontiguousarray(out, dtype=np.float32), res


LAST_POS = None


class Runner:
    """Jit the compiled Bass program once; support repeated timed runs.

    Mirrors bass2jax.run_bass_via_pjrt's multi-core path, but keeps the
    jitted callable and pre-placed device inputs so subsequent calls measure
    device execution without retrace/recompile or H2D of the big tensors.
    """

    def __init__(self, nc, in_maps, n_cores):
        import jax
        import jax.numpy as jnp
        from jax.sharding import Mesh, NamedSharding, PartitionSpec
        from jax.experimental.shard_map import shard_map

        from concourse import bass2jax as B2J
        from concourse import mybir as _mb

        B2J.install_neuronx_cc_hook()
        self.jax = jax
        partition_name = (
            nc.partition_id_tensor.name if nc.partition_id_tensor else None
        )
        in_names, out_names, out_avals, zero_outs = [], [], [], []
        for alloc in nc.m.functions[0].allocations:
            if not isinstance(alloc, _mb.MemoryLocationSet):
                continue
            name = alloc.memorylocations[0].name
            if alloc.kind == "ExternalInput":
                if name != partition_name:
                    in_names.append(name)
            elif alloc.kind == "ExternalOutput":
                shape = tuple(alloc.tensor_shape)
                dtype = _mb.dt.np(alloc.dtype)
                out_names.append(name)
                out_avals.append(jax.core.ShapedArray(shape, dtype))
                zero_outs.append(np.zeros(shape, dtype))
        n_params = len(in_names)
        all_in_names = list(in_names) + out_names
        if partition_name is not None:
            all_in_names.append(partition_name)
        donate = tuple(range(n_params, n_params + len(out_avals)))

        def _body(*args):
            operands = list(args)
            if partition_name is not None:
                operands.append(B2J.partition_id_tensor())
            outs = B2J._bass_exec_p.bind(
                *operands,
                out_avals=tuple(out_avals),
                in_names=tuple(all_in_names),
                out_names=tuple(out_names),
                lowering_input_output_aliases=(),
                sim_require_finite=True,
                sim_require_nnan=True,
                nc=nc,
            )
            return tuple(outs)

        devices = jax.devices()[:n_cores]
        mesh = Mesh(np.asarray(devices), ("core",))
        self.mesh = mesh
        spec = PartitionSpec("core")
        in_specs = (spec,) * (n_params + len(out_avals))
        out_specs = (spec,) * len(out_names)
        self.fn = jax.jit(
            shard_map(
                _body, mesh=mesh, in_specs=in_specs, out_specs=out_specs,
                check_rep=False,
            ),
            donate_argnums=donate,
            keep_unused=True,
        )
        sharding = NamedSharding(mesh, spec)
        concat_in = [
            np.concatenate([np.asarray(m[name]) for m in in_maps], axis=0)
            for name in in_names
        ]
        self.dev_in = [jax.device_put(a, sharding) for a in concat_in]
        self.zero_outs = zero_outs
        self.sharding = sharding
        self.out_names = out_names
        self.out_avals = out_avals
        self.n_cores = n_cores

    def _zeros(self):
        return [
            self.jax.device_put(
                np.zeros((self.n_cores * z.shape[0], *z.shape[1:]), z.dtype),
                self.sharding,
            )
            for z in self.zero_outs
        ]

    def run(self):
        outs = self.fn(*self.dev_in, *self._zeros())
        self.jax.block_until_ready(outs)
        return outs

    def timed(self, iters=20):
        import time

        zero_sets = [self._zeros() for _ in range(iters)]
        self.jax.block_until_ready(zero_sets)
        outs = None
        times = []
        for i in range(iters):
            t0 = time.perf_counter()
            outs = self.fn(*self.dev_in, *zero_sets[i])
            self.jax.block_until_ready(outs)
            times.append(time.perf_counter() - t0)
        return outs, times

    def results(self, outs):
        res = []
        for c in range(self.n_cores):
            res.append(
                {
                    name: np.asarray(outs[i]).reshape(
                        self.n_cores, *self.out_avals[i].shape
                    )[c]
                    for i, name in enumerate(self.out_names)
                }
            )
        return res



def make_runner(x, edge_index, Wl, bl, Wr, n_cores=8, **opts):
    """Build host data + compiled program + Runner (for timing loops)."""
    x = np.asarray(x, dtype=np.float32)
    saved = {}
    orig = bass_utils.run_bass_kernel_spmd

    def capture(nc, in_maps, core_ids, trace=False):
        saved["nc"], saved["in_maps"] = nc, in_maps
        raise _Captured()

    class _Captured(Exception):
        pass

    bass_utils.run_bass_kernel_spmd = capture
    try:
        run_graph(x, edge_index, Wl, bl, Wr, n_cores=n_cores, **opts)
    except _Captured:
        pass
    finally:
        bass_utils.run_bass_kernel_spmd = orig
    saved["pos"] = LAST_POS
    return Runner(saved["nc"], saved["in_maps"], n_cores), saved


def kernel(**inputs) -> np.ndarray:
    out, _ = run_graph(
        inputs["x"],
        inputs["edge_index"],
        inputs["Wl"],
        inputs["bl"],
        inputs["Wr"],
        n_cores=8,
    )
    return out

